# revision 23
# baseline (speedup 1.0000x reference)
"""Trainium2 Bass kernel for a transformer encoder layer.

B=4, S=2048, D=1024, H=16 heads (HD=64), PF=4096, fp32 I/O.

Sharding: 8 cores, core c handles batch c//2, query seq-half c%2 (1024
tokens). Each core computes K/V over its batch's full 2048-token sequence
(duplicated within the pair) so no collectives are needed.

v2 layout/precision strategy:
- srcT is pre-transposed host-side and shipped as fp8e4 [128, DK, S2]
  (own query half first, so queries are srcT cols 0:1024 on every core).
- QKV projections run fp8 DoubleRow (2 contraction elems/cell): weights
  prepacked [p, t, j, n] with d = 256t + 128j + p.
- K^T/Q^T evicted to bf16. Scores S^T = K Q^T computed per head PAIR with
  64-row tile_position packing: head A (KT rows 0:64) on tile (0,0), head
  B (rows 64:128) on tile (64,0) run concurrently.
- expS = exp(S^T/8) evicted by the scalar engine to fp8e4 (max 122 < 240),
  laid out [128, u, j, q] with key = 256u + 128j + p for DoubleRow PV.
- PV runs fp8 DoubleRow with the ones-row trick (M=65 -> denominator).
- Out-projection accumulates per-pair into an SBUF fp32 residual tile so
  its PE work and the DVE adds overlap the scalar exp stream.
- FFN stays bf16 (fp8 would exceed the accuracy budget). W1 loaded once.
- LN math in fp32. Attention-phase exp on the scalar engine is the
  binding resource; everything else is scheduled to hide under it.
"""

import numpy as np

D = 1024
S2 = 2048
SQ = 1024
PF = 4096
H = 16
HD = 64
DK = D // 128
PFK = PF // 128
NP = 8                 # head pairs; pair k = heads (2k, 2k+1) = D cols [128k, 128k+128)
VW = 68                # padded per-head V width (64 dims + ones row + 3 pad)
SCALE = 1.0 / 8.0
EPS = 1e-5
N_CORES = 8

_CACHE = {}


def _build():
    import concourse.bass as bass
    import concourse.mybir as mybir
    import concourse.tile as tile
    from concourse import bacc
    from concourse.masks import make_identity

    f32 = mybir.dt.float32
    bf16 = mybir.dt.bfloat16
    f8 = mybir.dt.float8e4
    AF = mybir.ActivationFunctionType
    ALU = mybir.AluOpType
    DR = mybir.MatmulPerfMode.DoubleRow

    nc = bacc.Bacc("TRN2", target_bir_lowering=False, debug=False, num_devices=N_CORES)

    def din(name, shape, dt=f32):
        return nc.dram_tensor(name, shape, dt, kind="ExternalInput")

    srcT = din("srcT", [128, DK, S2], f8)     # pre-transposed host-side
    src_qb = din("src_qb", [SQ, D])           # src_q + bo, pre-added host-side
    Wq = din("Wq", [D, D], f8)
    Wk = din("Wk", [D, D], f8)
    Wv = din("Wv", [D, D], f8)
    Wo = din("Wo", [D, D], bf16)
    W1 = din("W1", [D, PF], bf16)
    W2 = din("W2", [PF, D], bf16)
    bq = din("bq", [D])
    bk = din("bk", [D])
    bv = din("bv", [D])
    bf1 = din("bf1", [PF])
    bf2 = din("bf2", [D])
    g1 = din("g1", [D])
    b1 = din("b1", [D])
    g2 = din("g2", [D])
    b2 = din("b2", [D])
    out = nc.dram_tensor("out", [SQ, D], f32, kind="ExternalOutput")

    def bc_ap(vec, n):
        return bass.AP(tensor=vec, offset=0, ap=[[0, 128], [1, n]])

    def col_ap(vec, m):
        return bass.AP(tensor=vec, offset=0, ap=[[1, 128], [128, m]])

    with tile.TileContext(nc) as tc:
        import contextlib

        with contextlib.ExitStack() as ctx:
            consts = ctx.enter_context(tc.tile_pool(name="consts", bufs=1))

            identity = consts.tile([128, 128], f32)
            make_identity(nc, identity)
            id_bf = consts.tile([128, 128], bf16)
            nc.vector.tensor_copy(out=id_bf, in_=identity)

            bq_col = consts.tile([128, DK], f32)
            nc.sync.dma_start(out=bq_col, in_=col_ap(bq, DK))
            bk_col = consts.tile([128, DK], f32)
            nc.sync.dma_start(out=bk_col, in_=col_ap(bk, DK))
            bf1_col = consts.tile([128, PFK], f32)
            nc.sync.dma_start(out=bf1_col, in_=col_ap(bf1, PFK))


            eps_t = consts.tile([128, 1], f32)
            nc.vector.memset(eps_t, EPS)
            negc_t = consts.tile([128, 1], f32)
            nc.vector.memset(negc_t, -2.0)

            src1 = consts.tile([128, SQ // 128, D], bf16)  # LN1 out, 2MB
            # [p, m, a, c]: dim = a*128+p, token = m*128+c; per-m contiguous
            # so the XBAR DMA-transpose destination is legal
            src1T = consts.tile([128, SQ // 128, DK, 128], bf16)   # 2MB

            def layer_norm(r_row, g_bc, b_bc, out_tile, tmp_pool):
                stats = tmp_pool.tile([128, 2, 6], f32, tag="ln_stats")
                rr = r_row.rearrange("p (a f) -> p a f", a=2)
                for a in range(2):
                    nc.vector.bn_stats(out=stats[:, a, :], in_=rr[:, a, :])
                mv = tmp_pool.tile([128, 2], f32, tag="ln_mv")
                nc.vector.bn_aggr(out=mv, in_=stats)
                rstd = tmp_pool.tile([128, 1], f32, tag="ln_rstd")
                nc.scalar.activation(
                    out=rstd, in_=mv[:, 1:2], func=AF.Sqrt, bias=eps_t, scale=1.0
                )
                nc.vector.reciprocal_approx_fast(out=rstd, in_=rstd)
                nc.vector.scalar_tensor_tensor(
                    out=out_tile,
                    in0=r_row,
                    scalar=mv[:, 0:1],
                    in1=g_bc,
                    op0=ALU.subtract,
                    op1=ALU.mult,
                )
                nc.vector.scalar_tensor_tensor(
                    out=out_tile,
                    in0=out_tile,
                    scalar=rstd,
                    in1=b_bc,
                    op0=ALU.mult,
                    op1=ALU.add,
                )

            # ============ attention (per head pair, pipelined) ============
            with contextlib.ExitStack() as oattn:
                obig = oattn.enter_context(tc.tile_pool(name="obig", bufs=1))
                otmp = oattn.enter_context(tc.tile_pool(name="otmp", bufs=2))

                r_sb = obig.tile([128, SQ // 128, D], f32)  # residual accum, 4MB
                nc.sync.dma_start(
                    out=r_sb, in_=src_qb.rearrange("(m p) d -> p m d", p=128)
                )
                g1_bc = obig.tile([128, D], f32)
                nc.gpsimd.dma_start(out=g1_bc, in_=bc_ap(g1, D))
                b1_bc = obig.tile([128, D], f32)
                nc.gpsimd.dma_start(out=b1_bc, in_=bc_ap(b1, D))

                attn_ctx = oattn.enter_context(contextlib.ExitStack())
                qkps = attn_ctx.enter_context(
                    tc.tile_pool(name="qkps", bufs=1, space="PSUM")
                )
                projps = attn_ctx.enter_context(
                    tc.tile_pool(name="projps", bufs=1, space="PSUM")
                )
                pvps = attn_ctx.enter_context(
                    tc.tile_pool(name="pvps", bufs=1, space="PSUM")
                )
                abig = attn_ctx.enter_context(tc.tile_pool(name="abig", bufs=1))
                wst = attn_ctx.enter_context(tc.tile_pool(name="wst", bufs=1))
                ktp = attn_ctx.enter_context(tc.tile_pool(name="ktp", bufs=2))
                vtp = attn_ctx.enter_context(tc.tile_pool(name="vtp", bufs=1))
                expp = attn_ctx.enter_context(tc.tile_pool(name="expp", bufs=2))
                nrm = attn_ctx.enter_context(tc.tile_pool(name="nrm", bufs=2))
                wop = attn_ctx.enter_context(tc.tile_pool(name="wop", bufs=2))

                srcT_sb = abig.tile([128, DK, S2], f8)   # 2MB
                for tch in range(4):
                    nc.sync.dma_start(
                        out=srcT_sb[:, 2 * tch : 2 * tch + 2, :],
                        in_=srcT[:, 2 * tch : 2 * tch + 2, :],
                    )
                xts = abig.tile([128, DK, SQ], bf16)     # attn out x^T, 2MB
                bv_bc = abig.tile([128, D], f32)
                nc.gpsimd.dma_start(out=bv_bc, in_=bc_ap(bv, D))

                # PE warm-up with real matmuls (transposes don't engage HAM)
                for w in range(3):
                    wps = qkps.tile([128, 1024], f32, tag="qk", bufs=2)
                    for j in range(8):
                        nc.tensor.matmul(
                            wps[:, j * 128 : (j + 1) * 128], id_bf, id_bf,
                            start=True, stop=True,
                        )

                for k in range(NP):
                    c0 = k * 128
                    # -- fp8 weight slices for this pair (d = 256t + 128j + p) --
                    wk_s = wst.tile([128, 4, 2, 128], f8, tag="wk_s")
                    nc.sync.dma_start(
                        out=wk_s,
                        in_=Wk.rearrange("(t j p) n -> p t j n", p=128, j=2)[
                            :, :, :, c0 : c0 + 128
                        ],
                    )
                    wq_s = wst.tile([128, 4, 2, 128], f8, tag="wq_s")
                    nc.sync.dma_start(
                        out=wq_s,
                        in_=Wq.rearrange("(t j p) n -> p t j n", p=128, j=2)[
                            :, :, :, c0 : c0 + 128
                        ],
                    )

                    # -- KT_k [128, S2] bf16 --
                    KT = ktp.tile([128, S2], bf16, tag="KT")
                    for half in range(4):
                        ps = projps.tile([128, 512], f32, tag="proj", bufs=1)
                        for t in range(4):
                            nc.tensor.matmul(
                                ps,
                                wk_s[:, t, :, :],
                                srcT_sb[
                                    :, 2 * t : 2 * t + 2,
                                    half * 512 : (half + 1) * 512,
                                ],
                                start=(t == 0),
                                stop=(t == 3),
                                perf_mode=DR,
                            )
                        nc.vector.tensor_scalar_add(
                            out=KT[:, half * 512 : (half + 1) * 512],
                            in0=ps,
                            scalar1=bk_col[:, k : k + 1],
                        )

                    # -- QT_k [128, SQ] bf16 (queries are srcT cols 0:1024) --
                    QT = ktp.tile([128, SQ], bf16, tag="QT")
                    for hf in range(2):
                        ps = projps.tile([128, 512], f32, tag="proj", bufs=1)
                        for t in range(4):
                            nc.tensor.matmul(
                                ps,
                                wq_s[:, t, :, :],
                                srcT_sb[:, 2 * t : 2 * t + 2, hf * 512 : (hf + 1) * 512],
                                start=(t == 0),
                                stop=(t == 3),
                                perf_mode=DR,
                            )
                        nc.vector.tensor_scalar_add(
                            out=QT[:, hf * 512 : (hf + 1) * 512],
                            in0=ps, scalar1=bq_col[:, k : k + 1],
                        )

                    # -- V for a pair-pair (2 pairs at once), every other k --
                    if k % 2 == 0:
                        P2 = k // 2
                        wv_s = wst.tile([128, 4, 2, 256], f8, tag="wv_s")
                        nc.sync.dma_start(
                            out=wv_s,
                            in_=Wv.rearrange("(t j p) n -> p t j n", p=128, j=2)[
                                :, :, :, c0 : c0 + 256
                            ],
                        )
                        # V2 [128, u, j, 4 heads, 68] fp8, key = 256u + 128j + p
                        # (width padded 65->68 so the DoubleRow pairing stride
                        # 4*68=272 is 16-aligned; col 64 is the ones row)
                        V2 = vtp.tile([128, 8, 2, 4, VW], f8, tag="V2")
                        nc.vector.memset(V2[:, :, :, :, HD:VW], 1.0)
                        for uq in range(8):  # 2 ms blocks per psum
                            ps = projps.tile([128, 512], f32, tag="proj", bufs=1)
                            for j4 in range(2):
                                ms = uq * 2 + j4
                                for t in range(4):
                                    nc.tensor.matmul(
                                        ps[:, j4 * 256 : (j4 + 1) * 256],
                                        srcT_sb[
                                            :, 2 * t : 2 * t + 2,
                                            ms * 128 : (ms + 1) * 128,
                                        ],
                                        wv_s[:, t, :, :],
                                        start=(t == 0),
                                        stop=(t == 3),
                                        perf_mode=DR,
                                    )
                            for j4 in range(2):
                                ms = uq * 2 + j4
                                nc.vector.tensor_add(
                                    out=V2[:, ms // 2, ms % 2, :, 0:HD],
                                    in0=ps[
                                        :, j4 * 256 : (j4 + 1) * 256
                                    ].rearrange("p (h d) -> p h d", h=4),
                                    in1=bv_bc[:, P2 * 256 : (P2 + 1) * 256].rearrange(
                                        "p (h d) -> p h d", h=4
                                    ),
                                )

                    # -- scores + exp for both heads of the pair --
                    expA = expp.tile([128, 8, 2, SQ], f8, tag="expA")
                    expB = expp.tile([128, 8, 2, SQ], f8, tag="expB")
                    for sk in range(16):
                        u, jj = sk // 2, sk % 2
                        psA = qkps.tile([128, 1024], f32, tag="qk", bufs=2)
                        psB = qkps.tile([128, 1024], f32, tag="qk", bufs=2)
                        for sq in range(2):
                            nc.tensor.matmul(
                                psA[:, sq * 512 : (sq + 1) * 512],
                                KT[0:64, sk * 128 : (sk + 1) * 128],
                                QT[0:64, sq * 512 : (sq + 1) * 512],
                                start=True, stop=True,
                            )
                        for sq in range(2):
                            nc.tensor.matmul(
                                psB[:, sq * 512 : (sq + 1) * 512],
                                KT[64:128, sk * 128 : (sk + 1) * 128],
                                QT[64:128, sq * 512 : (sq + 1) * 512],
                                start=True, stop=True,
                            )
                        # exp(s/8 - 2): the global shift cancels in softmax
                        # and keeps expS well under the fp8e4 max of 240
                        nc.scalar.activation(
                            out=expA[:, u, jj, :], in_=psA, func=AF.Exp,
                            scale=SCALE, bias=negc_t,
                        )
                        nc.scalar.activation(
                            out=expB[:, u, jj, :], in_=psB, func=AF.Exp,
                            scale=SCALE, bias=negc_t,
                        )

                    # -- PV (fp8 DoubleRow, M=65 with ones-row denominator) --
                    V2v = None
                    for hh in range(2):
                        h_abs = 2 * k + hh
                        hl = h_abs % 4            # head slot within the pair-pair V2
                        expS = expA if hh == 0 else expB
                        pv = pvps.tile([VW, SQ], f32, tag="pv", bufs=1)
                        for u in range(8):
                            for sq in range(2):
                                nc.tensor.matmul(
                                    pv[:, sq * 512 : (sq + 1) * 512],
                                    V2[:, u, :, hl, :],
                                    expS[:, u, :, sq * 512 : (sq + 1) * 512],
                                    start=(u == 0),
                                    stop=(u == 7),
                                    perf_mode=DR,
                                )
                        den = nrm.tile([1, SQ], f32, tag="den", bufs=1)
                        nc.vector.tensor_copy(out=den, in_=pv[HD : HD + 1, :])
                        den_bc = nrm.tile([64, SQ], f32, tag="den_bc", bufs=1)
                        nc.gpsimd.partition_broadcast(den_bc, den)
                        nc.vector.reciprocal_approx_fast(out=den_bc, in_=den_bc)
                        xt = nrm.tile([64, SQ], bf16, tag="xt", bufs=1)
                        nc.vector.tensor_mul(out=xt, in0=pv[0:HD, :], in1=den_bc)
                        nc.sync.dma_start(
                            out=xts[hh * 64 : (hh + 1) * 64, k, :], in_=xt
                        )

                    # -- out-projection partial for this pair: r += x_k^T Wo_k --
                    wo_k = wop.tile([128, D], bf16, tag="wo_k")
                    nc.sync.dma_start(out=wo_k, in_=Wo[c0 : c0 + 128, :])
                    for m in range(SQ // 128):
                        for n in range(2):
                            po = projps.tile([128, 512], f32, tag="op", bufs=1)
                            nc.tensor.matmul(
                                po,
                                xts[:, k, m * 128 : (m + 1) * 128],
                                wo_k[:, n * 512 : (n + 1) * 512],
                                start=True, stop=True,
                            )
                            nc.vector.tensor_add(
                                out=r_sb[:, m, n * 512 : (n + 1) * 512],
                                in0=r_sb[:, m, n * 512 : (n + 1) * 512],
                                in1=po,
                            )
                        if k == NP - 1:
                            # LN1 (DVE/scalar only) right after the last
                            # pair's contribution to this token block
                            layer_norm(
                                r_sb[:, m, :], g1_bc, b1_bc, src1[:, m, :], otmp
                            )
                            # src1T via DMA XBAR transpose (frees the PE)
                            nc.sync.dma_start_transpose(
                                out=src1T[:, m, :, :],
                                in_=src1[:, m, :],
                            )

                # close inner attention pools (frees SBUF + PSUM)
                attn_ctx.close()

            # ============ src1T + FFN ============
            with contextlib.ExitStack() as fctx:
                ffps = fctx.enter_context(
                    tc.tile_pool(name="ffps", bufs=1, space="PSUM")
                )
                src1p = fctx.enter_context(tc.tile_pool(name="src1p", bufs=1))
                hpool = fctx.enter_context(tc.tile_pool(name="hpool", bufs=1))
                w2p = fctx.enter_context(tc.tile_pool(name="w2p", bufs=1))
                ftmp = fctx.enter_context(tc.tile_pool(name="ftmp", bufs=2))

                bf2_bc = src1p.tile([128, D], f32)
                nc.gpsimd.dma_start(out=bf2_bc, in_=bc_ap(bf2, D))
                g2_bc = src1p.tile([128, D], f32)
                nc.gpsimd.dma_start(out=g2_bc, in_=bc_ap(g2, D))
                b2_bc = src1p.tile([128, D], f32)
                nc.gpsimd.dma_start(out=b2_bc, in_=bc_ap(b2, D))

                hT = hpool.tile([128, PFK, SQ], bf16)          # 8MB
                w2sb = w2p.tile([128, PFK, D], bf16)           # 8MB

                for kb in range(8):
                    nc.sync.dma_start(
                        out=w2sb[:, kb * 4 : (kb + 1) * 4, :],
                        in_=W2.rearrange("(a p) n -> p a n", p=128)[
                            :, kb * 4 : (kb + 1) * 4, :
                        ],
                    )

                # FFN1: hT[pf, q] = relu(W1^T src1T + bf1); W1 loaded once
                for mp in range(PFK):
                    w1_s = ftmp.tile([128, DK, 128], bf16, tag="w1_s", bufs=4)
                    nc.sync.dma_start(
                        out=w1_s,
                        in_=W1.rearrange("(a p) n -> p a n", p=128)[
                            :, :, mp * 128 : (mp + 1) * 128
                        ],
                    )
                    for sqh in range(2):
                        ps = ffps.tile([128, 512], f32, tag="ff1", bufs=2)
                        for kd in range(DK):
                            nc.tensor.matmul(
                                ps,
                                w1_s[:, kd, :],
                                src1T[:, 4 * sqh : 4 * sqh + 4, kd, :],
                                start=(kd == 0),
                                stop=(kd == DK - 1),
                            )
                        nc.scalar.activation(
                            out=hT[:, mp, sqh * 512 : (sqh + 1) * 512],
                            in_=ps,
                            func=AF.Relu,
                            bias=bf1_col[:, mp : mp + 1],
                            scale=1.0,
                        )

                # FFN2 per m row + residual + LN2
                for m in range(SQ // 128):
                    ps = ffps.tile([128, 1024], f32, tag="ff2", bufs=2)
                    for kd in range(PFK):
                        for n in range(2):
                            nc.tensor.matmul(
                                ps[:, n * 512 : (n + 1) * 512],
                                hT[:, kd, m * 128 : (m + 1) * 128],
                                w2sb[:, kd, n * 512 : (n + 1) * 512],
                                start=(kd == 0),
                                stop=(kd == PFK - 1),
                            )
                    rr = ftmp.tile([128, D], f32, tag="rr", bufs=2)
                    nc.vector.tensor_add(out=rr, in0=ps, in1=src1[:, m, :])
                    nc.vector.tensor_add(out=rr, in0=rr, in1=bf2_bc)
                    layer_norm(rr, g2_bc, b2_bc, rr, ftmp)
                    nc.sync.dma_start(out=out[m * 128 : (m + 1) * 128, :], in_=rr)

    nc.compile()
    return nc


def make_in_maps(inputs):
    import ml_dtypes

    ins = {k: np.asarray(v, dtype=np.float32) for k, v in inputs.items()}
    bf = ml_dtypes.bfloat16
    f8 = ml_dtypes.float8_e4m3
    weights = {}
    for n in ["Wq", "Wk", "Wv"]:
        weights[n] = np.ascontiguousarray(ins[n]).astype(f8)
    for n in ["Wo", "W1", "W2"]:
        weights[n] = np.ascontiguousarray(ins[n]).astype(bf)
    for n in ["bq", "bk", "bv", "bf1", "bf2", "g1", "b1", "g2", "b2"]:
        weights[n] = np.ascontiguousarray(ins[n])

    src = ins["src"]
    in_maps = []
    for c in range(N_CORES):
        b, h = divmod(c, 2)
        m = dict(weights)
        sq = src[b, h * SQ : (h + 1) * SQ]          # own query half
        so = src[b, (1 - h) * SQ : (2 - h) * SQ]    # other half
        m["src_qb"] = np.ascontiguousarray(sq + ins["bo"][None, :])
        # srcT: [D, 2048] with own half first -> [128, DK, S2] fp8
        st = np.concatenate([sq, so], axis=0).T           # [D, S2]
        st = st.reshape(DK, 128, S2).transpose(1, 0, 2)   # [128, DK, S2]
        m["srcT"] = np.ascontiguousarray(st).astype(f8)
        in_maps.append(m)
    return in_maps


def kernel(**inputs):
    from concourse.bass_utils import run_bass_kernel_spmd

    if "nc" not in _CACHE:
        _CACHE["nc"] = _build()
    nc = _CACHE["nc"]

    in_maps = make_in_maps(inputs)
    res = run_bass_kernel_spmd(nc, in_maps, list(range(N_CORES)))

    outp = np.empty((4, S2, D), dtype=np.float32)
    for c in range(N_CORES):
        b, h = divmod(c, 2)
        outp[b, h * SQ : (h + 1) * SQ] = res.results[c]["out"]
    return outp


# revision 25
# speedup vs baseline: 1.0500x; 1.0500x over previous
"""Trainium2 Bass kernel for a transformer encoder layer.

B=4, S=2048, D=1024, H=16 heads (HD=64), PF=4096, fp32 I/O.

Sharding: 8 cores, core c handles batch c//2, query seq-half c%2 (1024
tokens). Each core computes K/V over its batch's full 2048-token sequence
(duplicated within the pair) so no collectives are needed.

v2 layout/precision strategy:
- srcT is pre-transposed host-side and shipped as fp8e4 [128, DK, S2]
  (own query half first, so queries are srcT cols 0:1024 on every core).
- QKV projections run fp8 DoubleRow (2 contraction elems/cell): weights
  prepacked [p, t, j, n] with d = 256t + 128j + p.
- K^T/Q^T evicted to bf16. Scores S^T = K Q^T computed per head PAIR with
  64-row tile_position packing: head A (KT rows 0:64) on tile (0,0), head
  B (rows 64:128) on tile (64,0) run concurrently.
- expS = exp(S^T/8) evicted by the scalar engine to fp8e4 (max 122 < 240),
  laid out [128, u, j, q] with key = 256u + 128j + p for DoubleRow PV.
- PV runs fp8 DoubleRow with the ones-row trick (M=65 -> denominator).
- Out-projection accumulates per-pair into an SBUF fp32 residual tile so
  its PE work and the DVE adds overlap the scalar exp stream.
- FFN stays bf16 (fp8 would exceed the accuracy budget). W1 loaded once.
- LN math in fp32. Attention-phase exp on the scalar engine is the
  binding resource; everything else is scheduled to hide under it.
"""

import numpy as np

D = 1024
S2 = 2048
SQ = 1024
PF = 4096
H = 16
HD = 64
DK = D // 128
PFK = PF // 128
NP = 8                 # head pairs; pair k = heads (2k, 2k+1) = D cols [128k, 128k+128)
VW = 68                # padded per-head V width (64 dims + ones row + 3 pad)
SCALE = 1.0 / 8.0
EPS = 1e-5
N_CORES = 8

_CACHE = {}


def _build():
    import concourse.bass as bass
    import concourse.mybir as mybir
    import concourse.tile as tile
    from concourse import bacc
    from concourse.masks import make_identity

    f32 = mybir.dt.float32
    bf16 = mybir.dt.bfloat16
    f8 = mybir.dt.float8e4
    AF = mybir.ActivationFunctionType
    ALU = mybir.AluOpType
    DR = mybir.MatmulPerfMode.DoubleRow

    nc = bacc.Bacc("TRN2", target_bir_lowering=False, debug=False, num_devices=N_CORES)

    def din(name, shape, dt=f32):
        return nc.dram_tensor(name, shape, dt, kind="ExternalInput")

    srcT = din("srcT", [128, DK, S2], f8)     # pre-transposed host-side
    src_qb = din("src_qb", [SQ, D])           # src_q + bo, pre-added host-side
    # prepacked host-side for contiguous DMA: [pair, p, t, j, cols]
    Wq = din("Wq", [NP, 128, 4, 2, 128], f8)
    Wk = din("Wk", [NP, 128, 4, 2, 128], f8)
    Wv = din("Wv", [4, 128, 4, 2, 256], f8)
    Wo = din("Wo", [D, D], bf16)
    W1 = din("W1", [PFK, 128, DK, 128], bf16)  # [mp, p, a, cols]
    W2 = din("W2", [PF, D], bf16)
    bq = din("bq", [D])
    bk = din("bk", [D])
    bv = din("bv", [D])
    bf1 = din("bf1", [PF])
    bf2 = din("bf2", [D])
    g1 = din("g1", [D])
    b1 = din("b1", [D])
    g2 = din("g2", [D])
    b2 = din("b2", [D])
    out = nc.dram_tensor("out", [SQ, D], f32, kind="ExternalOutput")

    def bc_ap(vec, n):
        return bass.AP(tensor=vec, offset=0, ap=[[0, 128], [1, n]])

    def col_ap(vec, m):
        return bass.AP(tensor=vec, offset=0, ap=[[1, 128], [128, m]])

    with tile.TileContext(nc) as tc:
        import contextlib

        with contextlib.ExitStack() as ctx:
            consts = ctx.enter_context(tc.tile_pool(name="consts", bufs=1))

            identity = consts.tile([128, 128], f32)
            make_identity(nc, identity)
            id_bf = consts.tile([128, 128], bf16)
            nc.vector.tensor_copy(out=id_bf, in_=identity)

            bq_col = consts.tile([128, DK], f32)
            nc.sync.dma_start(out=bq_col, in_=col_ap(bq, DK))
            bk_col = consts.tile([128, DK], f32)
            nc.sync.dma_start(out=bk_col, in_=col_ap(bk, DK))
            bf1_col = consts.tile([128, PFK], f32)
            nc.sync.dma_start(out=bf1_col, in_=col_ap(bf1, PFK))


            eps_t = consts.tile([128, 1], f32)
            nc.vector.memset(eps_t, EPS)
            negc_t = consts.tile([128, 1], f32)
            nc.vector.memset(negc_t, -2.0)

            src1 = consts.tile([128, SQ // 128, D], bf16)  # LN1 out, 2MB
            # [p, m, a, c]: dim = a*128+p, token = m*128+c; per-m contiguous
            # so the XBAR DMA-transpose destination is legal
            src1T = consts.tile([128, SQ // 128, DK, 128], bf16)   # 2MB

            def layer_norm(r_row, g_bc, b_bc, out_tile, tmp_pool):
                stats = tmp_pool.tile([128, 2, 6], f32, tag="ln_stats")
                rr = r_row.rearrange("p (a f) -> p a f", a=2)
                for a in range(2):
                    nc.vector.bn_stats(out=stats[:, a, :], in_=rr[:, a, :])
                mv = tmp_pool.tile([128, 2], f32, tag="ln_mv")
                nc.vector.bn_aggr(out=mv, in_=stats)
                rstd = tmp_pool.tile([128, 1], f32, tag="ln_rstd")
                nc.scalar.activation(
                    out=rstd, in_=mv[:, 1:2], func=AF.Sqrt, bias=eps_t, scale=1.0
                )
                nc.vector.reciprocal_approx_fast(out=rstd, in_=rstd)
                nc.vector.scalar_tensor_tensor(
                    out=out_tile,
                    in0=r_row,
                    scalar=mv[:, 0:1],
                    in1=g_bc,
                    op0=ALU.subtract,
                    op1=ALU.mult,
                )
                nc.vector.scalar_tensor_tensor(
                    out=out_tile,
                    in0=out_tile,
                    scalar=rstd,
                    in1=b_bc,
                    op0=ALU.mult,
                    op1=ALU.add,
                )

            # ============ attention (per head pair, pipelined) ============
            with contextlib.ExitStack() as oattn:
                obig = oattn.enter_context(tc.tile_pool(name="obig", bufs=1))
                otmp = oattn.enter_context(tc.tile_pool(name="otmp", bufs=2))

                r_sb = obig.tile([128, SQ // 128, D], f32)  # residual accum, 4MB
                nc.sync.dma_start(
                    out=r_sb, in_=src_qb.rearrange("(m p) d -> p m d", p=128)
                )
                g1_bc = obig.tile([128, D], f32)
                nc.gpsimd.dma_start(out=g1_bc, in_=bc_ap(g1, D))
                b1_bc = obig.tile([128, D], f32)
                nc.gpsimd.dma_start(out=b1_bc, in_=bc_ap(b1, D))

                attn_ctx = oattn.enter_context(contextlib.ExitStack())
                qkps = attn_ctx.enter_context(
                    tc.tile_pool(name="qkps", bufs=1, space="PSUM")
                )
                projps = attn_ctx.enter_context(
                    tc.tile_pool(name="projps", bufs=1, space="PSUM")
                )
                pvps = attn_ctx.enter_context(
                    tc.tile_pool(name="pvps", bufs=1, space="PSUM")
                )
                abig = attn_ctx.enter_context(tc.tile_pool(name="abig", bufs=1))
                wst = attn_ctx.enter_context(tc.tile_pool(name="wst", bufs=2))
                ktp = attn_ctx.enter_context(tc.tile_pool(name="ktp", bufs=2))
                vtp = attn_ctx.enter_context(tc.tile_pool(name="vtp", bufs=2))
                expp = attn_ctx.enter_context(tc.tile_pool(name="expp", bufs=2))
                nrm = attn_ctx.enter_context(tc.tile_pool(name="nrm", bufs=2))
                wop = attn_ctx.enter_context(tc.tile_pool(name="wop", bufs=2))
                abig2 = attn_ctx.enter_context(tc.tile_pool(name="abig2", bufs=2))

                srcT_sb = abig.tile([128, DK, S2], f8)   # 2MB
                for tch in range(4):
                    nc.sync.dma_start(
                        out=srcT_sb[:, 2 * tch : 2 * tch + 2, :],
                        in_=srcT[:, 2 * tch : 2 * tch + 2, :],
                    )
                bv_bc = abig.tile([128, D], bf16)
                nc.gpsimd.dma_start(out=bv_bc, in_=bc_ap(bv, D))

                # PE warm-up with real matmuls (transposes don't engage HAM)
                for w in range(3):
                    wps = qkps.tile([128, 1024], f32, tag="qk", bufs=2)
                    for j in range(8):
                        nc.tensor.matmul(
                            wps[:, j * 128 : (j + 1) * 128], id_bf, id_bf,
                            start=True, stop=True,
                        )

                def proj_kt_qt(k):
                    wk_s = wst.tile([128, 4, 2, 128], f8, tag="wk_s")
                    nc.sync.dma_start(out=wk_s, in_=Wk[k])
                    wq_s = wst.tile([128, 4, 2, 128], f8, tag="wq_s")
                    nc.sync.dma_start(out=wq_s, in_=Wq[k])

                    KT = ktp.tile([128, S2], bf16, tag="KT")
                    for half in range(4):
                        ps = projps.tile([128, 512], f32, tag="proj", bufs=1)
                        for t in range(4):
                            nc.tensor.matmul(
                                ps,
                                wk_s[:, t, :, :],
                                srcT_sb[
                                    :, 2 * t : 2 * t + 2,
                                    half * 512 : (half + 1) * 512,
                                ],
                                start=(t == 0),
                                stop=(t == 3),
                                perf_mode=DR,
                            )
                        nc.vector.tensor_scalar_add(
                            out=KT[:, half * 512 : (half + 1) * 512],
                            in0=ps,
                            scalar1=bk_col[:, k : k + 1],
                        )

                    QT = ktp.tile([128, SQ], bf16, tag="QT")
                    for hf in range(2):
                        ps = projps.tile([128, 512], f32, tag="proj", bufs=1)
                        for t in range(4):
                            nc.tensor.matmul(
                                ps,
                                wq_s[:, t, :, :],
                                srcT_sb[:, 2 * t : 2 * t + 2, hf * 512 : (hf + 1) * 512],
                                start=(t == 0),
                                stop=(t == 3),
                                perf_mode=DR,
                            )
                        nc.vector.tensor_scalar_add(
                            out=QT[:, hf * 512 : (hf + 1) * 512],
                            in0=ps, scalar1=bq_col[:, k : k + 1],
                        )
                    return KT, QT

                def proj_v(P2):
                    wv_s = wst.tile([128, 4, 2, 256], f8, tag="wv_s")
                    nc.sync.dma_start(out=wv_s, in_=Wv[P2])
                    # V2 [128, u, j, 4 heads, 68] fp8, key = 256u + 128j + p
                    # (width padded 65->68 so the DoubleRow pairing stride
                    # 4*68=272 is 16-aligned; col 64 is the ones row)
                    V2 = vtp.tile([128, 8, 2, 4, VW], f8, tag="V2")
                    nc.vector.memset(V2[:, :, :, :, HD:VW], 1.0)
                    for uq in range(8):  # 2 ms blocks per psum
                        ps = projps.tile([128, 512], f32, tag="proj", bufs=1)
                        for j4 in range(2):
                            ms = uq * 2 + j4
                            for t in range(4):
                                nc.tensor.matmul(
                                    ps[:, j4 * 256 : (j4 + 1) * 256],
                                    srcT_sb[
                                        :, 2 * t : 2 * t + 2,
                                        ms * 128 : (ms + 1) * 128,
                                    ],
                                    wv_s[:, t, :, :],
                                    start=(t == 0),
                                    stop=(t == 3),
                                    perf_mode=DR,
                                )
                        for j4 in range(2):
                            ms = uq * 2 + j4
                            nc.vector.tensor_add(
                                out=V2[:, ms // 2, ms % 2, :, 0:HD],
                                in0=ps[
                                    :, j4 * 256 : (j4 + 1) * 256
                                ].rearrange("p (h d) -> p h d", h=4),
                                in1=bv_bc[:, P2 * 256 : (P2 + 1) * 256].rearrange(
                                    "p (h d) -> p h d", h=4
                                ),
                            )
                    return V2

                def qk_exp(k, KT, QT):
                    expA = expp.tile([128, 8, 2, SQ], f8, tag="expA")
                    expB = expp.tile([128, 8, 2, SQ], f8, tag="expB")
                    for sk in range(16):
                        u, jj = sk // 2, sk % 2
                        psA = qkps.tile([128, 1024], f32, tag="qk", bufs=2)
                        psB = qkps.tile([128, 1024], f32, tag="qk", bufs=2)
                        for sq in range(2):
                            nc.tensor.matmul(
                                psA[:, sq * 512 : (sq + 1) * 512],
                                KT[0:64, sk * 128 : (sk + 1) * 128],
                                QT[0:64, sq * 512 : (sq + 1) * 512],
                                start=True, stop=True,
                            )
                            nc.tensor.matmul(
                                psB[:, sq * 512 : (sq + 1) * 512],
                                KT[64:128, sk * 128 : (sk + 1) * 128],
                                QT[64:128, sq * 512 : (sq + 1) * 512],
                                start=True, stop=True,
                            )
                        # exp(s/8 - 2): the global shift cancels in softmax
                        # and keeps expS well under the fp8e4 max of 240
                        nc.scalar.activation(
                            out=expA[:, u, jj, :], in_=psA, func=AF.Exp,
                            scale=SCALE, bias=negc_t,
                        )
                        nc.scalar.activation(
                            out=expB[:, u, jj, :], in_=psB, func=AF.Exp,
                            scale=SCALE, bias=negc_t,
                        )
                    return expA, expB

                def pv_oproj(k, V2, expA, expB):
                    xts_k = abig2.tile([128, SQ], bf16, tag="xts", bufs=2)
                    for hh in range(2):
                        h_abs = 2 * k + hh
                        hl = h_abs % 4        # head slot within the pair-pair V2
                        expS = expA if hh == 0 else expB
                        pv = pvps.tile([VW, SQ], f32, tag="pv", bufs=1)
                        for u in range(8):
                            for sq in range(2):
                                nc.tensor.matmul(
                                    pv[:, sq * 512 : (sq + 1) * 512],
                                    V2[:, u, :, hl, :],
                                    expS[:, u, :, sq * 512 : (sq + 1) * 512],
                                    start=(u == 0),
                                    stop=(u == 7),
                                    perf_mode=DR,
                                )
                        den = nrm.tile([1, SQ], f32, tag="den", bufs=1)
                        nc.vector.tensor_copy(out=den, in_=pv[HD : HD + 1, :])
                        den_bc = nrm.tile([64, SQ], f32, tag="den_bc", bufs=1)
                        nc.gpsimd.partition_broadcast(den_bc, den)
                        nc.vector.reciprocal_approx_fast(out=den_bc, in_=den_bc)
                        xt = nrm.tile([64, SQ], bf16, tag="xt", bufs=1)
                        nc.vector.tensor_mul(out=xt, in0=pv[0:HD, :], in1=den_bc)
                        nc.sync.dma_start(
                            out=xts_k[hh * 64 : (hh + 1) * 64, :], in_=xt
                        )

                    wo_k = wop.tile([128, D], bf16, tag="wo_k")
                    nc.sync.dma_start(out=wo_k, in_=Wo[k * 128 : (k + 1) * 128, :])
                    for m in range(SQ // 128):
                        for n in range(2):
                            po = projps.tile([128, 512], f32, tag="op", bufs=1)
                            nc.tensor.matmul(
                                po,
                                xts_k[:, m * 128 : (m + 1) * 128],
                                wo_k[:, n * 512 : (n + 1) * 512],
                                start=True, stop=True,
                            )
                            nc.vector.tensor_add(
                                out=r_sb[:, m, n * 512 : (n + 1) * 512],
                                in0=r_sb[:, m, n * 512 : (n + 1) * 512],
                                in1=po,
                            )
                        if k == NP - 1:
                            # LN1 (DVE/scalar only) right after the last
                            # pair's contribution to this token block
                            layer_norm(
                                r_sb[:, m, :], g1_bc, b1_bc, src1[:, m, :], otmp
                            )
                            # src1T via DMA XBAR transpose (frees the PE)
                            nc.sync.dma_start_transpose(
                                out=src1T[:, m, :, :],
                                in_=src1[:, m, :],
                            )

                # software pipeline: QK(k)+exp first each iteration (top
                # scheduler priority -> feeds the scalar engine), previous
                # pair's PV/oproj and next pair's projections as PE filler
                kt_cur = proj_kt_qt(0)
                v2_of = {0: proj_v(0)}
                state = {}
                for k in range(NP + 1):
                    if k < NP:
                        expAB = qk_exp(k, *kt_cur)
                        state[k] = (v2_of[k // 2], expAB)
                    if k >= 1:
                        V2p, (eA, eB) = state.pop(k - 1)
                        pv_oproj(k - 1, V2p, eA, eB)
                        if k - 1 == 1 or k - 1 == 3 or k - 1 == 5:
                            del v2_of[(k - 1) // 2]
                    if k < NP - 1:
                        kt_cur = proj_kt_qt(k + 1)
                    if k < NP - 2 and k % 2 == 0:
                        v2_of[k // 2 + 1] = proj_v(k // 2 + 1)

                # close inner attention pools (frees SBUF + PSUM)
                attn_ctx.close()

            # ============ src1T + FFN ============
            with contextlib.ExitStack() as fctx:
                ffps = fctx.enter_context(
                    tc.tile_pool(name="ffps", bufs=1, space="PSUM")
                )
                src1p = fctx.enter_context(tc.tile_pool(name="src1p", bufs=1))
                hpool = fctx.enter_context(tc.tile_pool(name="hpool", bufs=1))
                w2p = fctx.enter_context(tc.tile_pool(name="w2p", bufs=1))
                ftmp = fctx.enter_context(tc.tile_pool(name="ftmp", bufs=2))

                bf2_bc = src1p.tile([128, D], f32)
                nc.gpsimd.dma_start(out=bf2_bc, in_=bc_ap(bf2, D))
                g2_bc = src1p.tile([128, D], f32)
                nc.gpsimd.dma_start(out=g2_bc, in_=bc_ap(g2, D))
                b2_bc = src1p.tile([128, D], f32)
                nc.gpsimd.dma_start(out=b2_bc, in_=bc_ap(b2, D))

                hT = hpool.tile([128, PFK, SQ], bf16)          # 8MB
                w2sb = w2p.tile([128, PFK, D], bf16)           # 8MB

                for kb in range(8):
                    nc.sync.dma_start(
                        out=w2sb[:, kb * 4 : (kb + 1) * 4, :],
                        in_=W2.rearrange("(a p) n -> p a n", p=128)[
                            :, kb * 4 : (kb + 1) * 4, :
                        ],
                    )

                # FFN1: hT[pf, q] = relu(W1^T src1T + bf1); W1 loaded once
                for mp in range(PFK):
                    w1_s = ftmp.tile([128, DK, 128], bf16, tag="w1_s", bufs=4)
                    nc.sync.dma_start(out=w1_s, in_=W1[mp])
                    for sqh in range(2):
                        ps = ffps.tile([128, 512], f32, tag="ff1", bufs=2)
                        for kd in range(DK):
                            nc.tensor.matmul(
                                ps,
                                w1_s[:, kd, :],
                                src1T[:, 4 * sqh : 4 * sqh + 4, kd, :],
                                start=(kd == 0),
                                stop=(kd == DK - 1),
                            )
                        nc.scalar.activation(
                            out=hT[:, mp, sqh * 512 : (sqh + 1) * 512],
                            in_=ps,
                            func=AF.Relu,
                            bias=bf1_col[:, mp : mp + 1],
                            scale=1.0,
                        )

                # FFN2 per m row + residual + LN2
                for m in range(SQ // 128):
                    ps = ffps.tile([128, 1024], f32, tag="ff2", bufs=2)
                    for kd in range(PFK):
                        for n in range(2):
                            nc.tensor.matmul(
                                ps[:, n * 512 : (n + 1) * 512],
                                hT[:, kd, m * 128 : (m + 1) * 128],
                                w2sb[:, kd, n * 512 : (n + 1) * 512],
                                start=(kd == 0),
                                stop=(kd == PFK - 1),
                            )
                    rr = ftmp.tile([128, D], f32, tag="rr", bufs=2)
                    nc.vector.tensor_add(out=rr, in0=ps, in1=src1[:, m, :])
                    nc.vector.tensor_add(out=rr, in0=rr, in1=bf2_bc)
                    layer_norm(rr, g2_bc, b2_bc, rr, ftmp)
                    nc.sync.dma_start(out=out[m * 128 : (m + 1) * 128, :], in_=rr)

    nc.compile()
    return nc


def make_in_maps(inputs):
    import ml_dtypes

    ins = {k: np.asarray(v, dtype=np.float32) for k, v in inputs.items()}
    bf = ml_dtypes.bfloat16
    f8 = ml_dtypes.float8_e4m3
    weights = {}

    def pack_qk(w):  # [D, D] -> [NP, 128, 4, 2, 128]
        a = w.reshape(4, 2, 128, NP, 128).transpose(3, 2, 0, 1, 4)
        return np.ascontiguousarray(a).astype(f8)

    weights["Wq"] = pack_qk(ins["Wq"])
    weights["Wk"] = pack_qk(ins["Wk"])
    wv = ins["Wv"].reshape(4, 2, 128, 4, 256).transpose(3, 2, 0, 1, 4)
    weights["Wv"] = np.ascontiguousarray(wv).astype(f8)
    weights["Wo"] = np.ascontiguousarray(ins["Wo"]).astype(bf)
    w1 = ins["W1"].reshape(DK, 128, PFK, 128).transpose(2, 1, 0, 3)
    weights["W1"] = np.ascontiguousarray(w1).astype(bf)
    weights["W2"] = np.ascontiguousarray(ins["W2"]).astype(bf)
    for n in ["bq", "bk", "bv", "bf1", "bf2", "g1", "b1", "g2", "b2"]:
        weights[n] = np.ascontiguousarray(ins[n])

    src = ins["src"]
    in_maps = []
    for c in range(N_CORES):
        b, h = divmod(c, 2)
        m = dict(weights)
        sq = src[b, h * SQ : (h + 1) * SQ]          # own query half
        so = src[b, (1 - h) * SQ : (2 - h) * SQ]    # other half
        m["src_qb"] = np.ascontiguousarray(sq + ins["bo"][None, :])
        # srcT: [D, 2048] with own half first -> [128, DK, S2] fp8
        st = np.concatenate([sq, so], axis=0).T           # [D, S2]
        st = st.reshape(DK, 128, S2).transpose(1, 0, 2)   # [128, DK, S2]
        m["srcT"] = np.ascontiguousarray(st).astype(f8)
        in_maps.append(m)
    return in_maps


def kernel(**inputs):
    from concourse.bass_utils import run_bass_kernel_spmd

    if "nc" not in _CACHE:
        _CACHE["nc"] = _build()
    nc = _CACHE["nc"]

    in_maps = make_in_maps(inputs)
    res = run_bass_kernel_spmd(nc, in_maps, list(range(N_CORES)))

    outp = np.empty((4, S2, D), dtype=np.float32)
    for c in range(N_CORES):
        b, h = divmod(c, 2)
        outp[b, h * SQ : (h + 1) * SQ] = res.results[c]["out"]
    return outp
